# revision 1
# baseline (speedup 1.0000x reference)
"""TRN2 Bass kernel for channel cross-attention (XCA-style).

Math (per batch element b, matching the jax reference):
  qp = Wq q + bq ; kp = Wk k + bk           (1x1 convs, q/k: (192, 16384))
  qn = qp / max(||qp||_row, eps) ; kn likewise (L2 norm over the 16384 axis)
  A  = softmax_d(qn_c . kn_d * temp_h)       per head (6 heads x 32 ch)
  out = Wo (A (Wv v + bv)) + bo

Kernel strategy (one batch element per NeuronCore, 8 cores):
  Pass 1 streams q,k once: a stacked conv ([qp^T | kp^T] per 128-px chunk via a
  block-diagonal weight rhs), accumulating the raw cross-gram G = qp kp^T and
  per-channel sum-of-squares in PSUM across all 128 chunks. Norms, temperature
  and softmax are then applied on the tiny (192, 32) per-head logit blocks.
  The v path is folded: W_comb = Wo A_blockdiag Wv and
  b_comb = Wo A bv + bo are computed on-chip (192x192), so pass 2 is a single
  1x1 conv streaming v once: out = W_comb v + b_comb.

Matmuls run in float32r (fast fp32 mode, ~1e-3 rel err) by default.
"""

import numpy as np
from contextlib import ExitStack

import concourse.bass as bass
import concourse.tile as tile
from concourse import bacc, mybir
from concourse.bass_utils import run_bass_kernel_spmd

DIM = 192
HEADS = 6
CH = 32
HW = 16384
B = 8
EPS = 1e-12

PXT = 1024                # pixels per DMA tile
# last tiles smaller: shortens the serial pass-1 compute tail
TILE_SIZES = [1024] * 15 + [512, 512]
TILE_OFFS = [sum(TILE_SIZES[:i]) for i in range(len(TILE_SIZES))]
NPXT = len(TILE_SIZES)
CHUNK = 128               # pixels per matmul chunk (K of gram)
MMN = 512                 # max matmul free size (fp32 PSUM bank)

F32 = mybir.dt.float32
F32R = mybir.dt.float32r

MM_DT = F32R              # matmul operand dtype (F32R fast / F32 exact)
GRAM_N = 256              # padded gram free size (>=256 for f32r full speed)

_CACHE = {}


def _build():
    nc = bacc.Bacc("TRN2", target_bir_lowering=False, debug=False)

    # q/k carry a host-prepended ones row (bias folded into the conv matmul)
    q_d = nc.declare_dram_parameter("q", [DIM + 1, HW], MM_DT, isOutput=False)
    k_d = nc.declare_dram_parameter("k", [DIM + 1, HW], MM_DT, isOutput=False)
    v_d = nc.declare_dram_parameter("v", [DIM, HW], MM_DT, isOutput=False)
    # all weights packed into one (128, 2048) param -> one DMA
    wpk_d = nc.declare_dram_parameter("wpack", [128, 2048], MM_DT, isOutput=False)
    out_d = nc.declare_dram_parameter("out", [DIM, HW], F32, isOutput=True)

    with tile.TileContext(nc) as tc, ExitStack() as ctx:
        wp = ctx.enter_context(tc.tile_pool(name="weights", bufs=1))
        pp = ctx.enter_context(tc.tile_pool(name="post", bufs=1))
        vpool = ctx.enter_context(tc.tile_pool(name="v_res", bufs=1))

        KB = DIM + 1 - 128  # 65: rows of the second conv K-chunk
        wpk = wp.tile([128, 2048], MM_DT, tag="wpk")
        # conv weights (cols 0:1024) land first; the rest can trail the
        # first input tiles in the DMA queue
        nc.sync.dma_start(wpk[:, 0:1024], wpk_d[:, 0:1024])
        wq0 = wpk[:, 0:256]
        wk0 = wpk[:, 256:512]
        wq1 = wpk[0:KB, 512:768]
        wk1 = wpk[0:KB, 768:1024]
        wv_t = wpk[:, 1024:1216]
        wv_b = wpk[0:64, 1216:1408]
        woT_t = wpk[:, 1408:1600]
        woT_b = wpk[0:64, 1600:1792]
        bv_t = wpk[:, 1792:1793]
        bv_b = wpk[0:64, 1793:1794]
        bo_r = wpk[0:1, 1794:1986].bitcast(F32)
        tmp_r = wpk[0:1, 1986:1992].bitcast(F32)

        ones_col = wp.tile([128, 2], MM_DT, tag="ones_col")
        nc.vector.memset(ones_col[:].bitcast(F32), 1.0)
        # preload ACT tables for Sqrt/Exp during pass-1 (the first use of an
        # activation set pays a ~1.3us table load; keep it off the post chain)
        warm = wp.tile([1, 2], F32, tag="warm")
        nc.vector.memset(warm[:], 1.0)
        warm2 = wp.tile([1, 2], F32, tag="warm2")
        nc.scalar.sqrt(warm2[:], warm[:])
        nc.scalar.activation(warm2[:], warm[:], mybir.ActivationFunctionType.Exp)
        # per-channel temperature row tile (filled at pass-1 iter 1, after
        # the wpack part-B DMA that carries `temp` has been emitted)
        temp192 = wp.tile([1, DIM], F32, tag="temp192")

        run_q = pp.tile([128, DIM], F32, tag="run_q")
        run_k = pp.tile([128, DIM], F32, tag="run_k")
        v_tiles = []

        with tc.tile_pool(name="acc_psum", bufs=1, space="PSUM") as accp:
            # long-lived PSUM accumulators (one bank each)
            g_top = accp.tile([128, GRAM_N], F32, tag="g_top")
            g_bot = accp.tile([64, GRAM_N], F32, tag="g_bot")

            # ---------------- pass 1: stream q, k (and prefetch v) ---------
            with tc.tile_pool(name="p1_in", bufs=2) as inp, \
                 tc.tile_pool(name="p1_mid", bufs=4) as midp, \
                 tc.tile_pool(name="p1_psum", bufs=3, space="PSUM") as convp:
                for i in range(NPXT):
                    SZ = TILE_SIZES[i]
                    off = TILE_OFFS[i]
                    px = slice(off, off + SZ)
                    tA = inp.tile([128, PXT], MM_DT, tag="tA")
                    tB = inp.tile([KB, PXT], MM_DT, tag="tB")
                    tC = inp.tile([128, PXT], MM_DT, tag="tC")
                    tD = inp.tile([KB, PXT], MM_DT, tag="tD")
                    nc.sync.dma_start(tA[:, 0:SZ], q_d[0:128, px])
                    nc.sync.dma_start(tB[:, 0:SZ], q_d[128:DIM + 1, px])
                    if i == 0:
                        nc.sync.dma_start(wpk[:, 1024:2048], wpk_d[:, 1024:2048])
                    nc.sync.dma_start(tC[:, 0:SZ], k_d[0:128, px])
                    nc.sync.dma_start(tD[:, 0:SZ], k_d[128:DIM + 1, px])
                    if i == 1:
                        # temp repeated 32x per head (in*0 + bias fill); off
                        # the critical path, after wpack part B exists
                        for h in range(HEADS):
                            nc.scalar.activation(
                                temp192[0:1, h * CH:(h + 1) * CH],
                                wpk[0:1, 0:CH].bitcast(F32),
                                mybir.ActivationFunctionType.Identity,
                                bias=tmp_r[0:1, h:h + 1], scale=0.0)
                    if i == NPXT - 1:
                        # warm the sqrt act-table set while pass 1 drains
                        # (Copy doesn't evict; keeps the load off the post chain)
                        nc.scalar.sqrt(warm2[:], warm[:])

                    for j in range(SZ // CHUNK):
                        first = (i == 0 and j == 0)
                        last = (i == NPXT - 1 and j == SZ // CHUNK - 1)
                        cs = slice(j * CHUNK, (j + 1) * CHUNK)
                        # conv: out = [ones;q]^T [b;W^T], cols 192:256 zero-pad
                        qps = convp.tile([128, GRAM_N], F32, tag="qps")
                        nc.tensor.matmul(qps[:], tA[:, cs], wq0,
                                         start=True, stop=False)
                        nc.tensor.matmul(qps[:], tB[:, cs], wq1,
                                         start=False, stop=True)
                        kps = convp.tile([128, GRAM_N], F32, tag="kps")
                        nc.tensor.matmul(kps[:], tC[:, cs], wk0,
                                         start=True, stop=False)
                        nc.tensor.matmul(kps[:], tD[:, cs], wk1,
                                         start=False, stop=True)

                        # qp^T / kp^T chunks to SBUF (f32r rounding via ACT)
                        qT = midp.tile([128, DIM], MM_DT, tag="qT")
                        nc.scalar.copy(qT[:], qps[:, 0:DIM])
                        kT = midp.tile([128, GRAM_N], MM_DT, tag="kT")
                        nc.scalar.copy(kT[:], kps[:])
                        sq_q = midp.tile([128, DIM], F32, tag="sq_q")
                        nc.vector.tensor_mul(sq_q[:], qT[:], qT[:])
                        sq_k = midp.tile([128, DIM], F32, tag="sq_k")
                        nc.vector.tensor_mul(sq_k[:], kT[:, 0:DIM], kT[:, 0:DIM])
                        # running sums of squares: two parallel serial chains
                        # (q on DVE, k on gpsimd) so neither paces the loop
                        if first:
                            nc.vector.tensor_copy(run_q[:], sq_q[:])
                            nc.gpsimd.tensor_copy(run_k[:], sq_k[:])
                        else:
                            nc.vector.tensor_add(run_q[:], run_q[:], sq_q[:])
                            nc.gpsimd.tensor_add(run_k[:], run_k[:], sq_k[:])

                        # raw cross-gram accumulation (N padded to GRAM_N;
                        # pad cols of kT are exact zeros)
                        nc.tensor.matmul(g_top[:], qT[:, 0:128], kT[:],
                                         start=first, stop=last)
                        nc.tensor.matmul(g_bot[:], qT[:, 128:DIM], kT[:],
                                         start=first, stop=last)

                    # prefetch v for pass 2 (keeps the DMA queue saturated;
                    # v stays SBUF-resident until consumed)
                    vt = vpool.tile([128, SZ], MM_DT, tag=f"vt{i}")
                    nc.sync.dma_start(vt[:], v_d[0:128, px])
                    vb = vpool.tile([64, SZ], MM_DT, tag=f"vb{i}")
                    nc.sync.dma_start(vb[:], v_d[128:192, px])
                    v_tiles.append((vt, vb))

            # ---- norms + column-scaled gram (reads PSUM accumulators) ----
            run_sq_r = pp.tile([128, 2 * DIM], MM_DT, tag="run_sq_r")
            nc.scalar.copy(run_sq_r[:, 0:DIM], run_q[:])
            nc.vector.tensor_copy(run_sq_r[:, DIM:], run_k[:])
            with tc.tile_pool(name="ssq_psum", bufs=1, space="PSUM") as sspp:
                ssq = sspp.tile([128, 4], F32, tag="ssq")
                nc.tensor.matmul(ssq[:, 0:2], run_sq_r[:, 0:128], ones_col[:],
                                 start=True, stop=False)
                nc.tensor.matmul(ssq[0:64, 2:4], run_sq_r[:, 128:192],
                                 ones_col[:], start=False, stop=True)
                # rk in row form: row-ssq_k via ones contraction (fp32r ok:
                # N=192 is even; 4cyc/row but one-time)
                ssqk_row = sspp.tile([2, DIM], F32, tag="ssqk_row")
                nc.tensor.matmul(ssqk_row[:], ones_col[:],
                                 run_sq_r[:, DIM:2 * DIM], start=True, stop=True)
                ssq_sb = pp.tile([128, 4], F32, tag="ssq_sb")
                nc.scalar.sqrt(ssq_sb[:], ssq[:])       # q norms
                norms = pp.tile([128, 4], F32, tag="norms")
                nc.vector.tensor_scalar_max(norms[:], ssq_sb[:], EPS)
                rsq = pp.tile([128, 4], F32, tag="rsq")
                nc.vector.reciprocal(rsq[:], norms[:])
                # rsq col pairs: 0=rq[0:128], 2=rq[128:192]
                nrm_row = pp.tile([1, DIM], F32, tag="rowA")
                nc.scalar.sqrt(nrm_row[:], ssqk_row[0:1, 0:DIM])
            # preload the exp act-set now; the DVE/Pool chain below hides it
            nc.scalar.activation(warm2[:], warm[:],
                                 mybir.ActivationFunctionType.Exp)
            nrm2_row = pp.tile([1, DIM], F32, tag="rowB")
            nc.vector.tensor_scalar_max(nrm2_row[:], nrm_row[:], EPS)
            rk_row = pp.tile([1, DIM], F32, tag="rowA")
            nc.vector.reciprocal(rk_row[:], nrm2_row[:])
            rk2 = pp.tile([1, DIM], F32, tag="rowB")
            nc.vector.tensor_mul(rk2[:], rk_row[:], temp192[:])
            Bt = pp.tile([128, DIM], F32, tag="Bt")
            nc.gpsimd.partition_broadcast(Bt[:], rk2[:])

            Gs_t = pp.tile([128, DIM], F32, tag="Gs_t")
            nc.vector.tensor_mul(Gs_t[:], g_top[:, 0:DIM], Bt[:])
            Gs_b = pp.tile([64, DIM], F32, tag="Gs_b")
            nc.vector.tensor_mul(Gs_b[:], g_bot[:, 0:DIM], Bt[0:64, :])
        # acc_psum closed here - PSUM free for the small matmuls below

        # ---- compact per-head logits + softmax (SBUF only) ----
        C1 = pp.tile([128, CH], F32, tag="C1")
        C2 = pp.tile([64, CH], F32, tag="C2")
        for h in range(4):
            hs = slice(h * CH, (h + 1) * CH)
            nc.scalar.mul(C1[hs, :], Gs_t[hs, hs], rsq[hs, 0:1])
        for h in range(4, HEADS):
            ps = slice((h - 4) * CH, (h - 3) * CH)
            hs = slice(h * CH, (h + 1) * CH)
            nc.scalar.mul(C2[ps, :], Gs_b[ps, hs], rsq[ps, 2:3])

        E1 = pp.tile([128, CH], F32, tag="E1")
        den1 = pp.tile([128, 1], F32, tag="den1")
        nc.scalar.activation(E1[:], C1[:], mybir.ActivationFunctionType.Exp,
                             accum_out=den1[:])
        E2 = pp.tile([64, CH], F32, tag="E2")
        den2 = pp.tile([64, 1], F32, tag="den2")
        nc.scalar.activation(E2[:], C2[:], mybir.ActivationFunctionType.Exp,
                             accum_out=den2[:])
        rden1 = pp.tile([128, 1], F32, tag="rden1")
        nc.vector.reciprocal(rden1[:], den1[:])
        rden2 = pp.tile([64, 1], F32, tag="rden2")
        nc.vector.reciprocal(rden2[:], den2[:])

        # block-diagonal attention matrix A (rows scaled by 1/den)
        BD_t = pp.tile([128, DIM], F32, tag="BD_t")
        nc.vector.memset(BD_t[:], 0.0)
        BD_b = pp.tile([64, DIM], F32, tag="BD_b")
        nc.vector.memset(BD_b[:], 0.0)
        for h in range(4):
            hs = slice(h * CH, (h + 1) * CH)
            nc.scalar.mul(BD_t[hs, hs], E1[hs, :], rden1[hs, 0:1])
        for h in range(4, HEADS):
            ps = slice((h - 4) * CH, (h - 3) * CH)
            hs = slice(h * CH, (h + 1) * CH)
            nc.scalar.mul(BD_b[ps, hs], E2[ps, :], rden2[ps, 0:1])

        # ---- X1 = A^T Wo^T ; W_comb^T = Wv^T X1 ; b_row = bv^T X1 ----
        with tc.tile_pool(name="post_psum", bufs=1, space="PSUM") as ppp:
            X1t = ppp.tile([128, DIM], F32, tag="X1t")
            X1b = ppp.tile([64, DIM], F32, tag="X1b")
            nc.tensor.matmul(X1t[:], BD_t[:, 0:128], woT_t.bitcast(F32),
                             start=True, stop=False)
            nc.tensor.matmul(X1t[:], BD_b[:, 0:128], woT_b.bitcast(F32),
                             start=False, stop=True)
            nc.tensor.matmul(X1b[:], BD_t[:, 128:DIM], woT_t.bitcast(F32),
                             start=True, stop=False)
            nc.tensor.matmul(X1b[:], BD_b[:, 128:DIM], woT_b.bitcast(F32),
                             start=False, stop=True)
            X1t_sb = pp.tile([128, DIM], F32, tag="X1t_sb")
            nc.scalar.copy(X1t_sb[:], X1t[:])
            X1b_sb = pp.tile([64, DIM], F32, tag="X1b_sb")
            nc.scalar.copy(X1b_sb[:], X1b[:])

            # brow first: its bias-transpose DMAs then overlap the P matmuls
            brow = ppp.tile([1, DIM], F32, tag="brow")
            nc.tensor.matmul(brow[:], bv_t.bitcast(F32), X1t_sb[:], start=True, stop=False)
            nc.tensor.matmul(brow[:], bv_b.bitcast(F32), X1b_sb[:], start=False, stop=True)
            bc_row = pp.tile([1, DIM], F32, tag="bc_row")
            nc.vector.tensor_add(bc_row[:], brow[:], bo_r)
            bc_t = pp.tile([128, 1], F32, tag="bc_t")
            nc.sync.dma_start(bc_t[:], bc_row[0:1, 0:128])
            bc_b = pp.tile([64, 1], F32, tag="bc_b")
            nc.sync.dma_start(bc_b[:], bc_row[0:1, 128:192])

            Pt = ppp.tile([128, DIM], F32, tag="Pt")
            Pb = ppp.tile([64, DIM], F32, tag="Pb")
            nc.tensor.matmul(Pt[:], wv_t[:, 0:128].bitcast(F32), X1t_sb[:],
                             start=True, stop=False)
            nc.tensor.matmul(Pt[:], wv_b[:, 0:128].bitcast(F32), X1b_sb[:],
                             start=False, stop=True)
            nc.tensor.matmul(Pb[:], wv_t[:, 128:DIM].bitcast(F32), X1t_sb[:],
                             start=True, stop=False)
            nc.tensor.matmul(Pb[:], wv_b[:, 128:DIM].bitcast(F32), X1b_sb[:],
                             start=False, stop=True)

            wcT_t = pp.tile([128, DIM], MM_DT, tag="wcT_t")
            nc.scalar.copy(wcT_t[:], Pt[:])
            wcT_b = pp.tile([64, DIM], MM_DT, tag="wcT_b")
            nc.scalar.copy(wcT_b[:], Pb[:])

        # ---------------- pass 2: out = W_comb v + b_comb ----------------
        with tc.tile_pool(name="p2_out", bufs=3) as op_, \
             tc.tile_pool(name="p2_psum", bufs=3, space="PSUM") as opp:
            for i in [NPXT - 1, NPXT - 2] + list(range(NPXT - 2)):
                SZ = TILE_SIZES[i]
                off = TILE_OFFS[i]
                px = slice(off, off + SZ)
                vt, vb = v_tiles[i]
                os_t = op_.tile([128, SZ], F32, tag="os_t")
                os_b = op_.tile([64, SZ], F32, tag="os_b")
                for h in range(SZ // MMN):
                    ms = slice(h * MMN, (h + 1) * MMN)
                    o_t = opp.tile([128, MMN], F32, tag="o_t")
                    o_b = opp.tile([64, MMN], F32, tag="o_b")
                    nc.tensor.matmul(o_t[:], wcT_t[:, 0:128], vt[:, ms],
                                     start=True, stop=False)
                    nc.tensor.matmul(o_t[:], wcT_b[:, 0:128], vb[:, ms],
                                     start=False, stop=True)
                    nc.tensor.matmul(o_b[:], wcT_t[:, 128:DIM], vt[:, ms],
                                     start=True, stop=False)
                    nc.tensor.matmul(o_b[:], wcT_b[:, 128:DIM], vb[:, ms],
                                     start=False, stop=True)
                    nc.scalar.activation(os_t[:, ms], o_t[:],
                                         mybir.ActivationFunctionType.Identity,
                                         bias=bc_t[:])
                    nc.vector.tensor_scalar_add(os_b[:, ms], o_b[:], bc_b[:])
                nc.sync.dma_start(out_d[0:128, px], os_t[:, 0:SZ])
                nc.sync.dma_start(out_d[128:192, px], os_b[:, 0:SZ])

    nc.compile()
    return nc


def _get_nc():
    if "nc" not in _CACHE:
        _CACHE["nc"] = _build()
    return _CACHE["nc"]


def _make_in_maps(inputs):
    q = np.asarray(inputs["q"], dtype=np.float32)
    k = np.asarray(inputs["k"], dtype=np.float32)
    v = np.asarray(inputs["v"], dtype=np.float32)
    wq = np.asarray(inputs["wq"], dtype=np.float32)
    wk = np.asarray(inputs["wk"], dtype=np.float32)
    wv_ = np.asarray(inputs["wv"], dtype=np.float32)
    wo = np.asarray(inputs["wo"], dtype=np.float32)
    bq = np.asarray(inputs["bq"], dtype=np.float32)
    bk = np.asarray(inputs["bk"], dtype=np.float32)
    bv_ = np.asarray(inputs["bv"], dtype=np.float32)
    bo = np.asarray(inputs["bo"], dtype=np.float32)
    temp = np.asarray(inputs["temperature"], dtype=np.float32).reshape(1, HEADS)

    # conv rhs = [bias; W^T] (193, 192) split at row 128, cols padded to 256
    def conv_rhs(w, bias):
        aug = np.concatenate([bias.reshape(1, DIM), w.T], axis=0)  # (193, 192)
        pad = np.zeros((DIM + 1, 256), dtype=np.float32)
        pad[:, 0:DIM] = aug
        return pad[0:128], pad[128:]

    wq0, wq1 = conv_rhs(wq, bq)
    wk0, wk1 = conv_rhs(wk, bk)
    ones_row = np.ones((1, HW), dtype=np.float32)

    wpack = np.zeros((128, 2048), dtype=np.float32)
    wpack[:, 0:256] = wq0
    wpack[:, 256:512] = wk0
    wpack[0:65, 512:768] = wq1
    wpack[0:65, 768:1024] = wk1
    wpack[:, 1024:1216] = wv_[0:128]
    wpack[0:64, 1216:1408] = wv_[128:192]
    woT = wo.T
    wpack[:, 1408:1600] = woT[0:128]
    wpack[0:64, 1600:1792] = woT[128:192]
    wpack[:, 1792] = bv_[0:128]
    wpack[0:64, 1793] = bv_[128:192]
    wpack[0, 1794:1986] = bo
    wpack[0, 1986:1992] = temp.reshape(HEADS)

    shared = {"wpack": np.ascontiguousarray(wpack)}
    in_maps = []
    for b in range(B):
        m = dict(shared)
        m["q"] = np.ascontiguousarray(
            np.concatenate([ones_row, q[b].reshape(DIM, HW)], axis=0))
        m["k"] = np.ascontiguousarray(
            np.concatenate([ones_row, k[b].reshape(DIM, HW)], axis=0))
        m["v"] = np.ascontiguousarray(v[b].reshape(DIM, HW))
        in_maps.append(m)
    return in_maps


def _get_runner():
    """Compile once and cache a sharded-jit runner (run_bass_kernel_spmd
    rebuilds its jit closure per call, which re-traces every time)."""
    if "runner" in _CACHE:
        return _CACHE["runner"]
    import jax
    import jax.numpy as jnp
    from jax.sharding import Mesh, PartitionSpec
    from jax.experimental.shard_map import shard_map
    from concourse import bass2jax, mybir as mb
    from concourse.bass2jax import _bass_exec_p, partition_id_tensor

    bass2jax.install_neuronx_cc_hook()
    nc = _get_nc()

    partition_name = nc.partition_id_tensor.name if nc.partition_id_tensor else None
    in_names, out_names, out_avals = [], [], []
    for alloc in nc.m.functions[0].allocations:
        if not isinstance(alloc, mb.MemoryLocationSet):
            continue
        name = alloc.memorylocations[0].name
        if alloc.kind == "ExternalInput":
            if name != partition_name:
                in_names.append(name)
        elif alloc.kind == "ExternalOutput":
            out_names.append(name)
            out_avals.append(jax.core.ShapedArray(
                tuple(alloc.tensor_shape), mb.dt.np(alloc.dtype)))
    n_params = len(in_names)
    n_outs = len(out_avals)
    all_in_names = tuple(in_names + out_names +
                         ([partition_name] if partition_name else []))

    def _body(*args):
        operands = list(args)
        if partition_name is not None:
            operands.append(partition_id_tensor())
        return tuple(_bass_exec_p.bind(
            *operands,
            out_avals=tuple(out_avals),
            in_names=all_in_names,
            out_names=tuple(out_names),
            lowering_input_output_aliases=(),
            sim_require_finite=True,
            sim_require_nnan=True,
            nc=nc,
        ))

    devices = jax.devices()[:B]
    mesh = Mesh(np.asarray(devices), ("core",))
    in_specs = (PartitionSpec("core"),) * (n_params + n_outs)
    out_specs = (PartitionSpec("core"),) * n_outs
    donate = tuple(range(n_params, n_params + n_outs))
    sharded = jax.jit(
        shard_map(_body, mesh=mesh, in_specs=in_specs, out_specs=out_specs,
                  check_rep=False),
        donate_argnums=donate, keep_unused=True)

    zero_shapes = [(B * a.shape[0], *a.shape[1:]) for a in out_avals]
    zero_dtypes = [a.dtype for a in out_avals]

    def run(in_maps):
        concat_in = [
            np.concatenate([np.asarray(in_maps[c][nm]) for c in range(B)], axis=0)
            for nm in in_names
        ]
        zeros = [jnp.zeros(s, d) for s, d in zip(zero_shapes, zero_dtypes)]
        outs = sharded(*concat_in, *zeros)
        return {
            nm: np.asarray(outs[i]).reshape(B, *out_avals[i].shape)
            for i, nm in enumerate(out_names)
        }

    _CACHE["runner"] = run
    return run


def _prebuild():
    """Compile the NEFF and warm the jit at import time so the first real
    kernel() call doesn't pay the ~10s build; never let this break import."""
    try:
        run = _get_runner()
        z = np.zeros((DIM + 1, HW), dtype=np.float32)
        zv = np.zeros((DIM, HW), dtype=np.float32)
        zw = np.zeros((128, 2048), dtype=np.float32)
        run([{"q": z, "k": z, "v": zv, "wpack": zw} for _ in range(B)])
    except Exception:
        _CACHE.clear()


def kernel(q, k, v, wq, bq, wk, bk, wv, bv, wo, bo, temperature):
    run = _get_runner()
    in_maps = _make_in_maps(dict(q=q, k=k, v=v, wq=wq, bq=bq, wk=wk, bk=bk,
                                 wv=wv, bv=bv, wo=wo, bo=bo,
                                 temperature=temperature))
    out = run(in_maps)["out"].reshape(B, DIM, 128, 128)
    return np.ascontiguousarray(out.astype(np.float32))


import os as _os
if not _os.environ.get("KERNEL_NO_PREBUILD"):
    _prebuild()

